# revision 33
# baseline (speedup 1.0000x reference)
"""Trainium2 Bass kernel for hierarchical-classification AWX head.

Computes, for inputs x[B, L] (f32) and 0/1 adjacency R[C, L] (int32):

    o   = sigmoid(x)
    s   = einsum('bl,cl->bc', o**5, R)          (R**5 == R since R is 0/1)
    out = clip(s, EPS, 1-EPS) ** (1/5)

Sharding: R is split row-wise (class dim) across the 8 NeuronCores; each
core computes a [B, C/8] slice of the output against the full (replicated)
x. No cross-device reduction is needed; the host concatenates the slices.

The host marshals both operands into the exact SBUF layout the PE wants
(contraction dim l on partitions), so the device does no transposes and
no PSUM-evacuation copies at all:
  - x -> [128, 2048] fp8_e4m3 with x_sb[p, 64k+b] = x[b, 128k+p]
    (|dx| <= 3.1% of |x| -> ~0.25% on s after random-sign averaging,
    well inside the 2e-2 output tolerance)
  - R -> [128, 8192] fp8_e4m3 (0/1 is exact) with
    r_sb[p, 256k+c] = R[c0+c, 128k+p]
This keeps combined DMA read+write bytes at ~2.6 MiB per core (the 16
shared SWDGE engines move ~435-455 GB/s combined r+w, the measured
bottleneck of the original revision at 6.5 MiB).

Per-core pipeline:
  - SWDGE queue: x halves first (they gate the activation front), then
    R in chunks; the tail chunks are finer so the last matmuls are not
    gated on one big completion.
  - o^5 split across engines so it hides under the R DMA: ScalarE runs
    a single table-based Sigmoid pass per column half; VectorE then
    squares twice and multiplies (o2 = o*o, o4 = o2*o2 in bf16 at DVE
    2x 16-bit rate, o5 = o4*o written as fp8 for the matmul). Measured
    alternatives that lost: all-ScalarE exp/ln/exp (3 serial ACT passes,
    blows past the DMA shadow) and DVE RECIPROCAL (microcoded, ~26x a
    multiply).
  - Two ACT table sets are pinned (build-time patch): sigmoid_and_others
    for the chain, natural_log_exp_and_others for the tail; the second
    ACT_TABLE_LOAD is queued after the chain and hides under the matmul
    phase.
  - 16 fp8 DoubleRow matmuls (2 k-chunks per instruction, 2x PE rate)
    accumulate s[64, 256] in a single PSUM bank. Dummy matmuls on a
    memset tile run from the start to warm the PE HAM clock gate, with
    fills sprinkled to bridge gaps.
  - Tail: clip (VectorE two-op tensor_scalar), ln, exp(0.2*) (ScalarE),
    out over the scalar-engine HWDGE ring.
"""

import numpy as np

B, L, C = 64, 4096, 2048
NCORES = 8
CP = C // NCORES  # 256 classes per core
EPS = 1e-6

NK = L // 128   # 32 contraction chunks of 128
NG = NK // 2    # 16 DoubleRow groups
XW = NK * B     # 2048 columns of marshaled x
RW = NK * CP    # 8192 columns of marshaled R
N_WARMUP_MM = 48

# R DMA chunk widths (columns of the marshaled [128, RW] layout); last
# chunks finer so the final matmul groups start sooner.
R_CHUNKS = [2048, 2048, 2048, 1024, 1024]

ACT_SETS = ("sigmoid_and_others", "natural_log_exp_and_others")

_STATE = {}


def _patch_act_tables():
    """Pin bacc's ACT table-set selection to exactly two sets: Sigmoid
    for the o^5 chain and Ln/Exp for the s^(1/5) tail, so the kernel
    pays exactly two ACT_TABLE_LOADs (the second hides under the matmul
    phase). Entry order and count are preserved so act_func_set_id stays
    aligned with the compiler's act_info.json."""
    import functools

    import concourse.bacc as bacc_mod
    import concourse.hw_specs as hw_specs

    if getattr(bacc_mod.get_activation_tables, "_awx_patched", False):
        return

    orig = hw_specs.get_activation_tables

    @functools.cache
    def patched(module_arch):
        tabs = orig(module_arch)
        for s in ACT_SETS:
            assert s in tabs, sorted(tabs)
        return {
            name: (fns if name in ACT_SETS else type(fns)())
            for name, fns in tabs.items()
        }

    patched._awx_patched = True
    bacc_mod.get_activation_tables = patched


def _build_nc():
    from contextlib import ExitStack

    import concourse.bacc as bacc
    import concourse.mybir as mybir
    from concourse.tile import TileContext

    _patch_act_tables()

    dt = mybir.dt
    AF = mybir.ActivationFunctionType
    ALU = mybir.AluOpType
    PM = mybir.MatmulPerfMode

    nc = bacc.Bacc("TRN2", target_bir_lowering=False)

    # One dram tensor per DMA chunk, each fully contiguous, so every
    # transfer is a single linear descriptor run (single_packet).
    x_ds = [
        nc.dram_tensor(f"x{h}", [128, XW // 2], dt.float8e4, kind="ExternalInput")
        for h in range(2)
    ]
    r_ds = [
        nc.dram_tensor(f"r{ci}", [128, wdt], dt.float8e4, kind="ExternalInput")
        for ci, wdt in enumerate(R_CHUNKS)
    ]
    o_d = nc.dram_tensor("out", [B, CP], dt.float32, kind="ExternalOutput")

    with TileContext(nc) as tc, ExitStack() as ctx:
        const = ctx.enter_context(tc.tile_pool(name="const", bufs=1))
        xin = ctx.enter_context(tc.tile_pool(name="xin", bufs=1))
        actp = ctx.enter_context(tc.tile_pool(name="actp", bufs=2))
        o5p = ctx.enter_context(tc.tile_pool(name="o5p", bufs=1))
        rbp = ctx.enter_context(tc.tile_pool(name="rbp", bufs=len(R_CHUNKS)))
        tailp = ctx.enter_context(tc.tile_pool(name="tailp", bufs=3))
        psw = ctx.enter_context(tc.tile_pool(name="psw", bufs=1, space="PSUM"))
        pssA = ctx.enter_context(tc.tile_pool(name="pssA", bufs=1, space="PSUM"))
        pssB = ctx.enter_context(tc.tile_pool(name="pssB", bufs=1, space="PSUM"))

        # PE warmup operand: memset (no DMA dependency) so dummy matmuls
        # start as soon as the engines come up, ramping the HAM clock gate.
        warm_mm = const.tile([128, 128], dt.bfloat16)
        nc.vector.memset(warm_mm[:], 0.0)

        # SWDGE stream: x halves first (they gate the activation front),
        # then R. (x on the scalar HWDGE queue was tried and reverted:
        # that ring moves only ~75 GB/s, landing x1 ~2 us late.)
        xf = xin.tile([128, XW], dt.float8e4)
        nc.gpsimd.dma_start(
            out=xf[:, : XW // 2], in_=x_ds[0][:], single_packet=True
        )
        nc.gpsimd.dma_start(
            out=xf[:, XW // 2 :], in_=x_ds[1][:], single_packet=True
        )

        rb = []          # (tile, start_col) per chunk
        col = 0
        for ci, wdt in enumerate(R_CHUNKS):
            t = rbp.tile([128, wdt], dt.float8e4, tag=f"rb{ci}")
            nc.gpsimd.dma_start(out=t[:], in_=r_ds[ci][:], single_packet=True)
            rb.append((t, col))
            col += wdt

        ps_w = psw.tile([128, 128], dt.float32)

        def fill(n):
            for _ in range(n):
                nc.tensor.matmul(
                    out=ps_w[:], lhsT=warm_mm[:], rhs=warm_mm[:],
                    start=True, stop=True,
                )

        fill(N_WARMUP_MM)

        # o5 = sigmoid(x)^5: one ScalarE Sigmoid pass per half (bf16),
        # then o2, o4, o5 multiplies on VectorE (all DVE TT runs at 1x;
        # the fp8-out o5 passes are ~1.8x a bf16 pass). Everything stays
        # on VectorE: a GpSimd variant was tried and reverted — DVE and
        # GpSimd share SBUF ports, so concurrent elementwise work slowed
        # both engines.
        o5b = o5p.tile([128, XW], dt.float8e4)
        HW_ = XW // 2

        def emit_chain(h):
            sl = slice(HW_ * h, HW_ * (h + 1))
            o = actp.tile([128, HW_], dt.bfloat16, tag="o")
            nc.scalar.activation(out=o[:], in_=xf[:, sl], func=AF.Sigmoid)
            o2 = actp.tile([128, HW_], dt.bfloat16, tag="o2")
            if h == 0:
                nc.vector.tensor_tensor(
                    out=o2[:], in0=o[:], in1=o[:], op=ALU.mult
                )
            else:
                # ScalarE is idle after the two Sigmoid passes; doing
                # h1's square there shortens the serial VectorE stream
                # that gates the last matmul groups.
                nc.scalar.activation(out=o2[:], in_=o[:], func=AF.Square)
            o4 = actp.tile([128, HW_], dt.bfloat16, tag="o4")
            nc.vector.tensor_tensor(out=o4[:], in0=o2[:], in1=o2[:], op=ALU.mult)
            if h == 0:
                # fp8 output costs VectorE ~1.8x a bf16 pass but ScalarE
                # nothing: produce h0's o5 in bf16 on VectorE and convert
                # on ScalarE (Copy is in every ACT table set, so this
                # cannot force an early table switch). h1's o5 goes fp8
                # directly on VectorE: routing its conversion through
                # ScalarE would serialize after the last V pass instead
                # of overlapping earlier ones.
                o5h = actp.tile([128, HW_], dt.bfloat16, tag="o5h")
                nc.vector.tensor_tensor(
                    out=o5h[:], in0=o4[:], in1=o[:], op=ALU.mult
                )
                nc.scalar.copy(out=o5b[:, sl], in_=o5h[:])
            else:
                nc.vector.tensor_tensor(
                    out=o5b[:, sl], in0=o4[:], in1=o[:], op=ALU.mult
                )

        # One PSUM tile per class-column half: the accumulation groups
        # must live in distinct PSUM zero regions (2 KiB banks).
        s_psA = pssA.tile([B, CP // 2], dt.float32)
        s_psB = pssB.tile([B, CP // 2], dt.float32)
        s_ps = [s_psA, s_psB]

        def chunk_of(g):
            c0 = 512 * g
            for t, start in rb:
                if start <= c0 < start + t.shape[1]:
                    return t, c0 - start
            raise AssertionError(g)

        def emit_mm(g, cs):
            # DoubleRow: contract chunks k=2g,2g+1 in one instruction,
            # for the class-column slice cs (accumulation regions of the
            # two column halves are address-disjoint in PSUM, so each
            # half carries its own start/stop pair and the first half's
            # tail overlaps the second half's matmuls).
            lhsT = o5b[:, 128 * g : 128 * (g + 1)].rearrange(
                "p (two b) -> p two b", two=2
            )
            t, off = chunk_of(g)
            rhs = t[:, off : off + 512].rearrange("p (two c) -> p two c", two=2)[
                :, :, 128 * cs : 128 * (cs + 1)
            ]
            nc.tensor.matmul(
                out=s_ps[cs][:],
                lhsT=lhsT,
                rhs=rhs,
                start=(g == 0),
                stop=(g == NG - 1),
                perf_mode=PM.DoubleRow,
            )

        # Tail per column half: out = exp(0.2 * ln(clip(s, EPS, 1-EPS))).
        # Half A's clip/ln/exp overlap half B's matmuls; both halves
        # write one SBUF tile, shipped by a single GpSimd-triggered DMA.
        ob = tailp.tile([B, CP], dt.float32)

        def emit_tail(cs):
            csl = slice(cs * (CP // 2), (cs + 1) * (CP // 2))
            s_sb = tailp.tile([B, CP // 2], dt.float32, tag=f"tail{cs}")
            nc.vector.tensor_scalar(
                out=s_sb[:],
                in0=s_ps[cs][:],
                scalar1=EPS,
                scalar2=1.0 - EPS,
                op0=ALU.max,
                op1=ALU.min,
            )
            w = tailp.tile([B, CP // 2], dt.float32, tag=f"tail{cs}")
            nc.scalar.activation(out=w[:], in_=s_sb[:], func=AF.Ln)
            nc.scalar.activation(
                out=ob[:, csl], in_=w[:], func=AF.Exp, scale=1.0 / 5.0
            )
            if cs == 1:
                nc.gpsimd.dma_start(out=o_d[:], in_=ob[:], single_packet=True)

        for h in range(2):
            emit_chain(h)
            for cs in range(2):
                for g in range(8 * h, 8 * h + 8):
                    emit_mm(g, cs)
                if h == 1:
                    emit_tail(cs)
            fill(12)

    nc.finalize()
    return nc


def _marshal_x(x: np.ndarray) -> np.ndarray:
    """[B, L] f32 -> [128, NK*B] fp8e4m3 with x_sb[p, 64k+b] = x[b, 128k+p]."""
    import ml_dtypes

    xt = np.ascontiguousarray(x, dtype=np.float32).T  # [L, B]
    xm = xt.reshape(NK, 128, B).transpose(1, 0, 2).reshape(128, XW)
    return np.ascontiguousarray(xm).astype(ml_dtypes.float8_e4m3fn)


def _marshal_r(Rs: np.ndarray) -> np.ndarray:
    """[CP, L] 0/1 int -> [128, NK*CP] fp8e4m3 with
    r_sb[p, 256k+c] = R[c, 128k+p]. 1.0 in e4m3 is 0x38, so the cast is
    a pure integer scale+view (exact)."""
    import ml_dtypes

    r8 = (Rs.astype(np.uint8) * np.uint8(0x38)).T  # [L, CP] bytes
    rm = r8.reshape(NK, 128, CP).transpose(1, 0, 2).reshape(128, RW)
    return np.ascontiguousarray(rm).view(ml_dtypes.float8_e4m3fn)


def kernel(inputs: np.ndarray, R: np.ndarray) -> np.ndarray:
    from concourse.bass_utils import run_bass_kernel_spmd

    if "nc" not in _STATE:
        _STATE["nc"] = _build_nc()
    nc = _STATE["nc"]

    xm = _marshal_x(inputs)
    xs = {
        f"x{h}": np.ascontiguousarray(xm[:, h * (XW // 2) : (h + 1) * (XW // 2)])
        for h in range(2)
    }
    in_maps = []
    for i in range(NCORES):
        rm = _marshal_r(R[i * CP : (i + 1) * CP])
        m = dict(xs)
        col = 0
        for ci, wdt in enumerate(R_CHUNKS):
            m[f"r{ci}"] = np.ascontiguousarray(rm[:, col : col + wdt])
            col += wdt
        in_maps.append(m)
    res = run_bass_kernel_spmd(nc, in_maps, core_ids=list(range(NCORES)))
    _STATE["last_results"] = res
    out = np.concatenate([res.results[i]["out"] for i in range(NCORES)], axis=1)
    return np.ascontiguousarray(out, dtype=np.float32)
